# revision 21
# baseline (speedup 1.0000x reference)
"""CstLoss on Trainium2 — self-contained Bass/Tile SPMD kernel (8 NeuronCores).

Reference math (per [N=64, C=17, H=128, W=128] f32 pair output/target):
  h/w marginal means of each map -> softmax over the 128-axis -> l2
  normalize -> sim_pos = mean of matched-channel cosines, sim = sum of
  mean-over-batch all-pairs cosines, loss = -log(sim_pos/sim)/C/N.

Key algebra:
  * softmax denominator AND the max-subtraction cancel under l2
    normalization (S/W stays in [-0.7, 0.7] here, so exp needs no
    stabilization), so each projection only needs e = exp(S/W) and
    q = e/||e||; the reference's 1e-8 norm clamp never binds.
  * ||e||^2 = sum exp(2S/W) (second Exp pass + accum_out) and
    1/||e|| = exp(-0.5 ln ||e||^2), so with Copy the ACT engine only uses
    functions from ONE table set (natural_log_exp_and_others; the greedy
    set selector is steered there by _patch_act_tables) -> a single
    ACT_TABLE_LOAD, fully hidden under the DMA window.
  * sum_ij dot(qo_i, qt_j) = dot(sum_i qo_i, sum_j qt_j): the CxC pair
    matrix is never materialized. The per-map 1/||e|| factors are folded
    into the channel-membership matrices (g0, gt) and the matched-dot
    scalars; one [128x16x256] matmul per tensor yields both segments'
    channel sums (cross-segment junk is masked before the B reduction).
  * on-device reduction to 3 scalars per core (A = matched-cosine sums per
    segment, B = sum_n U.V); host all-reduces and takes the log.

Schedule: per-engine FIFO order arranged by expected data readiness; sync
queue: o chunks, the two small-descriptor tail DMAs, then t chunks. DMA
partition dim must be exactly 128 (anything else collapses the descriptor
spray onto 4 SDMA engines). Early chunks are 32 h-rows (16KB descriptors,
which avoid the SDMA-engine-15 slow-descriptor penalty); t tapers to
16/8/4/4-row chunks so the final DVE reduce off the last bytes is short.
No SWDGE/gpsimd DMAs; consts and the tail scatter ride the scalar HWDGE
queue. Per chunk: DVE segmented reduce (row sums) + PE transpose-
accumulate (col sums); all PSUM->SBUF copies are ACT Copy; the o-tensor's
softmax, its U-channel sums, and all tail work overlap t's DMA window.
"""

import contextlib
import ctypes
import sys
import types
from contextlib import ExitStack

import numpy as np

import concourse.bacc as bacc
import concourse.hw_specs as hw_specs
import concourse.tile as tile
from concourse import mybir
from concourse.bass_utils import run_bass_kernel_spmd

F32 = mybir.dt.float32
AX = mybir.AxisListType
ACT = mybir.ActivationFunctionType

N, C, H, W = 64, 17, 128, 128
NCORES = 8
NLOC = N // NCORES           # 8 batch entries per core
MAPS = NLOC * C              # 136 maps per tensor per core
MAIN = 128                   # maps in the main batch
TAIL = MAPS - MAIN           # 8 maps in the tail
ROWS_O = [16, 16, 32, 32, 32]          # small first chunks: early DVE start
ROWS_T = [32, 32, 16, 16, 16, 8, 4, 2, 2]  # taper: tiny final reduces


def _patch_act_tables():
    """Steer the greedy ACT table-set selector to the one set that holds
    every function this kernel uses (exp, ln, copy), so only one
    ACT_TABLE_LOAD is ever emitted."""
    orig = hw_specs.get_activation_tables
    strip = {ACT.Exp, ACT.Ln, ACT.Copy, ACT.Identity}

    def patched(arch):
        tabs = orig(arch)
        return {
            name: (fns if name == "natural_log_exp_and_others" else fns - strip)
            for name, fns in tabs.items()
        }

    bacc.get_activation_tables = patched


def _install_ntff_hook():
    """Provide antenv.axon_hooks if the image lacks it (needed only when
    run_bass_kernel_spmd is called with trace=True; harmless otherwise)."""
    if "antenv.axon_hooks" in sys.modules:
        return
    so_path = "/opt/axon/libaxon_pjrt.so"
    hook = None
    try:
        lib = ctypes.CDLL(so_path)
        if hasattr(lib, "axon_start_nrt_profile"):
            lib.axon_start_nrt_profile.argtypes = [
                ctypes.POINTER(ctypes.c_int64),
                ctypes.c_size_t,
            ]
            lib.axon_start_nrt_profile.restype = ctypes.c_int64
            lib.axon_stop_nrt_profile.argtypes = [ctypes.c_char_p]
            lib.axon_stop_nrt_profile.restype = ctypes.c_int64

            @contextlib.contextmanager
            def _hook(output_dir, device_ids):
                import jax

                jax.devices()
                if device_ids:
                    ids = (ctypes.c_int64 * len(device_ids))(*device_ids)
                    rc = lib.axon_start_nrt_profile(ids, len(device_ids))
                else:
                    rc = lib.axon_start_nrt_profile(None, 0)
                if rc != 0:
                    raise RuntimeError(f"axon_start_nrt_profile rc={rc}")
                try:
                    yield
                finally:
                    n = lib.axon_stop_nrt_profile(str(output_dir).encode())
                    print(f"profile: {n} file(s) in {output_dir}", file=sys.stderr)

            hook = _hook
    except OSError:
        pass
    mod = types.ModuleType("antenv.axon_hooks")
    mod.get_axon_ntff_profile_hook = lambda: hook
    mod.set_axon_ntff_profile_hook = lambda h: None
    sys.modules["antenv.axon_hooks"] = mod


_patch_act_tables()
_install_ntff_hook()


def _body(tc, o_d, t_d, id_d, g0_d, gt_d, on_d, res_d):
    nc = tc.nc
    with ExitStack() as ctx:
        consts = ctx.enter_context(tc.tile_pool(name="consts", bufs=1))
        chunks = ctx.enter_context(tc.tile_pool(name="chunks", bufs=1))
        tailp = ctx.enter_context(tc.tile_pool(name="tailp", bufs=1))
        projp = ctx.enter_context(tc.tile_pool(name="projp", bufs=1))
        workp = ctx.enter_context(tc.tile_pool(name="workp", bufs=1))
        outp = ctx.enter_context(tc.tile_pool(name="outp", bufs=1))
        # PSUM: 8 distinct tiles = 8 banks, no slot rotation (slot reuse
        # with concurrent PE traffic wedges the device: NRT status 101).
        accps = ctx.enter_context(tc.tile_pool(name="accps", bufs=1, space="PSUM"))

        # ---- consts on the scalar HWDGE queue ----
        ident = consts.tile([128, 128], F32)
        nc.scalar.dma_start(ident[:], id_d)
        g0 = consts.tile([MAIN, NLOC], F32)
        nc.scalar.dma_start(g0[:], g0_d)
        gt = consts.tile([TAIL, NLOC], F32)
        nc.scalar.dma_start(gt[:], gt_d)
        ones = consts.tile([128, 1], F32)
        nc.scalar.dma_start(ones[:], on_d)

        # ---- sync-queue DMAs: o chunks, tails, t chunks ----
        chunk_tiles = {0: [], 1: []}
        r0s = {0: 0, 1: 0}

        def issue_chunks(ti, x_d, rows, lo, hi):
            for c in range(lo, hi):
                r = rows[c]
                r0 = r0s[ti]
                ck = chunks.tile([MAIN, r * W], F32, name=f"chunk{ti}_{c}")
                nc.sync.dma_start(ck[:], x_d[0:MAIN, r0 : r0 + r, :])
                chunk_tiles[ti].append((ck, r0, r))
                r0s[ti] += r

        tail2d = tailp.tile([128, 2 * TAIL * W], F32)
        tv = tail2d.rearrange("p (m w) -> p m w", w=W)
        nc.sync.dma_start(tv[:, 0:TAIL, :], o_d[MAIN:MAPS].rearrange("m h w -> h m w"))
        nc.sync.dma_start(
            tv[:, TAIL : 2 * TAIL, :], t_d[MAIN:MAPS].rearrange("m h w -> h m w")
        )
        issue_chunks(0, o_d, ROWS_O, 0, len(ROWS_O))
        issue_chunks(1, t_d, ROWS_T, 0, len(ROWS_T))

        # PSUM tiles (8 banks)
        wt_o = accps.tile([128, MAIN], F32)
        wt_t = accps.tile([128, MAIN], F32)
        wb_o = accps.tile([MAIN, 128], F32)
        wb_t = accps.tile([MAIN, 128], F32)
        tlA = accps.tile([65, 512], F32)
        tlB = accps.tile([65, 512], F32)
        # U8/V8: per-n channel sums, h-segment in cols 0:128, w-segment in
        # cols 128:256. U8 also hosts A in [0:2, 256] and B in [0:1, 257].
        U8 = accps.tile([NLOC, 512], F32)
        V8 = accps.tile([NLOC, 512], F32)

        proj_o = projp.tile([MAIN, 2 * W], F32)
        proj_t = projp.tile([MAIN, 2 * W], F32)
        eo = projp.tile([MAIN, 2 * W], F32)
        et = projp.tile([MAIN, 2 * W], F32)
        To = tailp.tile([TAIL, 2 * W], F32)
        Tt = tailp.tile([TAIL, 2 * W], F32)
        To_e = tailp.tile([TAIL, 2 * W], F32)
        Tt_e = tailp.tile([TAIL, 2 * W], F32)
        # ssq/rn cols: 0:2 = o(h,w), 2:4 = tail-o, 4:6 = tail-t, 6:8 = t
        ssq = workp.tile([128, 8], F32)
        lssq = workp.tile([128, 8], F32)
        rn = workp.tile([128, 8], F32)
        dump = workp.tile([MAIN, 2 * W], F32)
        dumpP = workp.tile([MAIN, 2 * W], F32)
        dumpT = tailp.tile([TAIL, 2 * W], F32)
        dumpS = tailp.tile([TAIL, 2 * W], F32)

        def seg_exp(P, proj, e, dmp, col):
            """e = exp(S/W) for one 128-wide segment; ssq col = sum exp(2S/W)."""
            nc.scalar.activation(e, proj, ACT.Exp, scale=1.0 / W)
            nc.scalar.activation(
                dmp, proj, ACT.Exp, scale=2.0 / W,
                accum_out=ssq[0:P, col : col + 1],
            )

        def do_chunk(ti, proj, wt, c, act_red=False):
            ck, r0, r = chunk_tiles[ti][c]
            cv = ck.rearrange("p (h w) -> p h w", w=W)
            nc.vector.reduce_sum(proj[:, r0 : r0 + r], cv, axis=AX.X)
            nch = len(chunk_tiles[ti])
            for j in range(r):
                nc.tensor.matmul(
                    wt[:],
                    cv[:, j, :],
                    ident[:],
                    is_transpose=True,
                    start=(c == 0 and j == 0),
                    stop=(c == nch - 1 and j == r - 1),
                )

        # tail w-marginals: four [1,512] ones-matmul column-sum rows
        tl_rows = [(tlA, 32, 0), (tlA, 64, 1), (tlB, 32, 2), (tlB, 64, 3)]
        for tl, base, kk in tl_rows:
            nc.tensor.matmul(
                tl[base : base + 1, :],
                ones[:],
                tail2d[:, kk * 512 : (kk + 1) * 512],
                skip_group_check=True,
            )
        R = tailp.tile([128, 2 * TAIL], F32)
        nc.vector.reduce_sum(R[:], tv, axis=AX.X)
        for i, (T, tl) in enumerate(((To, tlA), (Tt, tlB))):
            nc.tensor.matmul(
                tl[0:TAIL, 0:128],
                R[:, i * TAIL : (i + 1) * TAIL],
                ident[:],
                is_transpose=True,
                skip_group_check=True,
            )
            nc.scalar.copy(T[:, 0:W], tl[0:TAIL, 0:128])
        srow = tailp.tile([65, 512 * 4], F32)
        for tl, base, kk in tl_rows:
            nc.scalar.copy(srow[base : base + 1, kk * 512 : (kk + 1) * 512],
                           tl[base : base + 1, :])
        scat = [(To, 0, 32, 0), (To, 4, 64, 1), (Tt, 0, 32, 2), (Tt, 4, 64, 3)]
        for T, m0, base, kk in scat:
            nc.scalar.dma_start(
                T[m0 : m0 + 4, W : 2 * W],
                srow[base : base + 1, kk * 512 : (kk + 1) * 512],
            )
        for i, (T, T_e) in enumerate(((To, To_e), (Tt, Tt_e))):
            for s in range(2):
                seg_exp(
                    TAIL, T[:, s * W : (s + 1) * W], T_e[:, s * W : (s + 1) * W],
                    dumpT[:, s * W : (s + 1) * W], 2 + 2 * i + s,
                )

        # tail rn (cols 2:6), matched dots and g-folds — all in the DVE
        # bubbles of o's arrival-paced phase
        nc.scalar.activation(lssq[:, 2:6], ssq[:, 2:6], ACT.Ln)
        nc.scalar.activation(rn[:, 2:6], lssq[:, 2:6], ACT.Exp, scale=-0.5)
        post_seg = outp.tile([TAIL, 2], F32)
        nc.vector.tensor_mul(dumpS[:], To_e[:], Tt_e[:])
        nc.vector.reduce_sum(
            post_seg[:], dumpS.rearrange("p (s w) -> p s w", w=W), axis=AX.X
        )
        nc.vector.tensor_mul(post_seg[:], post_seg[:], rn[0:TAIL, 2:4])
        nc.vector.tensor_mul(post_seg[:], post_seg[:], rn[0:TAIL, 4:6])
        gtc_o = outp.tile([TAIL, 2 * NLOC], F32)
        gtc_t = outp.tile([TAIL, 2 * NLOC], F32)
        for s in range(2):
            nc.vector.tensor_scalar_mul(
                gtc_o[:, s * NLOC : (s + 1) * NLOC], gt[:], rn[0:TAIL, 2 + s : 3 + s]
            )
            nc.vector.tensor_scalar_mul(
                gtc_t[:, s * NLOC : (s + 1) * NLOC], gt[:], rn[0:TAIL, 4 + s : 5 + s]
            )

        # ---- all of o: chunks, then softmax-exp (tail data arrives after
        # o's bytes, so all tail compute comes after o's in engine FIFOs) ----
        for c in range(len(chunk_tiles[0])):
            do_chunk(0, proj_o, wt_o, c)
        seg_exp(MAIN, proj_o[:, 0:W], eo[:, 0:W], dump[:, 0:W], 0)
        wts_o = workp.tile([128, MAIN], F32)
        nc.scalar.copy(wts_o[:], wt_o[:])
        nc.tensor.matmul(wb_o[:], wts_o[:], ident[:], is_transpose=True)
        nc.scalar.copy(proj_o[:, W : 2 * W], wb_o[:])
        seg_exp(MAIN, proj_o[:, W : 2 * W], eo[:, W : 2 * W], dump[:, W : 2 * W], 1)

        # ---- o/tail finish, all hidden before t's compute ----
        # rn for o (cols 0:2; tail cols were finished early)
        nc.scalar.activation(lssq[:, 0:2], ssq[:, 0:2], ACT.Ln)
        nc.scalar.activation(rn[:, 0:2], lssq[:, 0:2], ACT.Exp, scale=-0.5)
        gsc_o = outp.tile([MAIN, 2 * NLOC], F32)
        for s in range(2):
            nc.vector.tensor_scalar_mul(
                gsc_o[:, s * NLOC : (s + 1) * NLOC], g0[:], rn[0:MAIN, s : s + 1]
            )
        for s in range(2):
            nc.tensor.matmul(
                U8[:, s * W : (s + 1) * W], gsc_o[:, s * NLOC : (s + 1) * NLOC],
                eo[:, s * W : (s + 1) * W],
                start=True, stop=False, skip_group_check=True)
            nc.tensor.matmul(
                U8[:, s * W : (s + 1) * W], gtc_o[:, s * NLOC : (s + 1) * NLOC],
                To_e[:, s * W : (s + 1) * W],
                start=False, stop=True, skip_group_check=True)
        Us = outp.tile([NLOC, 2 * W], F32)
        nc.scalar.copy(Us[:], U8[:, 0 : 2 * W])

        # ---- t chunks (the 8/4-row chunks' row-sums go to the otherwise
        # idle GPSIMD engine: when engine-15's DMA backlog releases the last
        # sems in a burst, DVE and GPSIMD drain the reduces in parallel) ----
        for c in range(len(chunk_tiles[1])):
            do_chunk(1, proj_t, wt_t, c)

        # ---- t finish: w-segment exp first (wb_t is ready before the
        # last h-row reduces land); one merged Ln/exp pair then produces
        # both rn columns, after which all scale-dependent work runs ----
        wts_t = workp.tile([128, MAIN], F32)
        nc.scalar.copy(wts_t[:], wt_t[:])
        pos_seg = outp.tile([MAIN, 2], F32)
        gsc_t = outp.tile([MAIN, 2 * NLOC], F32)
        nc.tensor.matmul(wb_t[:], wts_t[:], ident[:], is_transpose=True)
        nc.scalar.copy(proj_t[:, W : 2 * W], wb_t[:])
        seg_exp(MAIN, proj_t[:, W : 2 * W], et[:, W : 2 * W], dump[:, W : 2 * W], 7)
        nc.vector.tensor_mul(dumpP[:, W : 2 * W], eo[:, W : 2 * W], et[:, W : 2 * W])
        nc.vector.reduce_sum(pos_seg[:, 1:2], dumpP[:, W : 2 * W], axis=AX.X)
        seg_exp(MAIN, proj_t[:, 0:W], et[:, 0:W], dump[:, 0:W], 6)
        nc.vector.tensor_mul(dumpP[:, 0:W], eo[:, 0:W], et[:, 0:W])
        nc.vector.reduce_sum(pos_seg[:, 0:1], dumpP[:, 0:W], axis=AX.X)
        nc.scalar.activation(lssq[:, 6:8], ssq[:, 6:8], ACT.Ln)
        nc.scalar.activation(rn[:, 6:8], lssq[:, 6:8], ACT.Exp, scale=-0.5)
        nc.vector.tensor_mul(pos_seg[:], pos_seg[:], rn[0:MAIN, 0:2])
        nc.vector.tensor_mul(pos_seg[:], pos_seg[:], rn[0:MAIN, 6:8])
        for s in range(2):
            nc.vector.tensor_scalar_mul(
                gsc_t[:, s * NLOC : (s + 1) * NLOC], g0[:], rn[0:MAIN, 6 + s : 7 + s]
            )
            nc.tensor.matmul(
                V8[:, s * W : (s + 1) * W], gsc_t[:, s * NLOC : (s + 1) * NLOC],
                et[:, s * W : (s + 1) * W],
                start=True, stop=False, skip_group_check=True,
            )
            nc.tensor.matmul(
                V8[:, s * W : (s + 1) * W], gtc_t[:, s * NLOC : (s + 1) * NLOC],
                Tt_e[:, s * W : (s + 1) * W],
                start=False, stop=True, skip_group_check=True,
            )

        # A (per-segment matched-cosine sums) into U16[0:2, 256]
        nc.tensor.matmul(
            U8[0:2, 256:257], pos_seg[:], ones[0:MAIN, 0:1],
            start=True, stop=False, skip_group_check=True,
        )
        nc.tensor.matmul(
            U8[0:2, 256:257], post_seg[:], ones[0:TAIL, 0:1],
            start=False, stop=True, skip_group_check=True,
        )

        # B = sum_n U.V into U8[0:1, 257]: one full-row reduce covers both
        # segments (no junk in the [8, 256] layout)
        uv = outp.tile([NLOC, 2 * W], F32)
        nc.vector.tensor_mul(uv[:], Us[:], V8[:, 0 : 2 * W])
        uvs = outp.tile([NLOC, 1], F32)
        nc.vector.reduce_sum(uvs[:], uv[:], axis=AX.X)
        nc.tensor.matmul(
            U8[0:1, 257:258], uvs[:], ones[0:NLOC, 0:1], skip_group_check=True
        )

        res_s = outp.tile([2, 2], F32)
        nc.scalar.copy(res_s[:], U8[0:2, 256:258])
        nc.sync.dma_start(res_d, res_s[:])


def _build_nc():
    nc = bacc.Bacc("TRN2", target_bir_lowering=False, debug=False)
    o_d = nc.dram_tensor("o", [MAPS, H, W], F32, kind="ExternalInput").ap()
    t_d = nc.dram_tensor("t", [MAPS, H, W], F32, kind="ExternalInput").ap()
    id_d = nc.dram_tensor("ident", [128, 128], F32, kind="ExternalInput").ap()
    g0_d = nc.dram_tensor("g0", [MAIN, NLOC], F32, kind="ExternalInput").ap()
    gt_d = nc.dram_tensor("gt", [TAIL, NLOC], F32, kind="ExternalInput").ap()
    on_d = nc.dram_tensor("ones", [128, 1], F32, kind="ExternalInput").ap()
    res_d = nc.dram_tensor("res", [2, 2], F32, kind="ExternalOutput").ap()
    with tile.TileContext(nc) as tc:
        _body(tc, o_d, t_d, id_d, g0_d, gt_d, on_d, res_d)
    nc.compile()
    return nc


_NC = None


def _get_nc():
    global _NC
    if _NC is None:
        _NC = _build_nc()
    return _NC


_IDENT = np.eye(128, dtype=np.float32)
_G0 = np.zeros((MAIN, NLOC), np.float32)
_G0[np.arange(MAIN), np.arange(MAIN) // C] = 1.0
_GT = np.zeros((TAIL, NLOC), np.float32)
_GT[np.arange(TAIL), (MAIN + np.arange(TAIL)) // C] = 1.0
_ONES = np.ones((128, 1), np.float32)


def _make_in_maps(output, target):
    in_maps = []
    for i in range(NCORES):
        o = np.ascontiguousarray(output[i * NLOC : (i + 1) * NLOC]).reshape(MAPS, H, W)
        t = np.ascontiguousarray(target[i * NLOC : (i + 1) * NLOC]).reshape(MAPS, H, W)
        in_maps.append(
            {"o": o, "t": t, "ident": _IDENT, "g0": _G0, "gt": _GT,
             "ones": _ONES}
        )
    return in_maps


def _finish(results):
    A = 0.0
    B = 0.0
    for r in results:
        res = r["res"].astype(np.float64)
        A += res[0, 0] + res[1, 0]
        B += res[0, 1]
    # sim_pos = 0.5*A/(N*C); sim = 0.5*B/N; loss = -log(sim_pos/sim)/(C*N)
    loss = -np.log(A / (C * B)) / (C * N)
    return np.float32(loss)


def kernel(output, target):
    output = np.asarray(output, dtype=np.float32)
    target = np.asarray(target, dtype=np.float32)
    nc = _get_nc()
    res = run_bass_kernel_spmd(nc, _make_in_maps(output, target), list(range(NCORES)))
    return _finish(res.results)


def profile(output, target):
    """Run once with NTFF tracing; returns max per-core HW exec time in ns."""
    output = np.asarray(output, dtype=np.float32)
    target = np.asarray(target, dtype=np.float32)
    nc = _get_nc()
    res = run_bass_kernel_spmd(
        nc, _make_in_maps(output, target), list(range(NCORES)), trace=True
    )
    return res.exec_time_ns


# revision 22
# speedup vs baseline: 1.0191x; 1.0191x over previous
"""CstLoss on Trainium2 — self-contained Bass/Tile SPMD kernel (8 NeuronCores).

Reference math (per [N=64, C=17, H=128, W=128] f32 pair output/target):
  h/w marginal means of each map -> softmax over the 128-axis -> l2
  normalize -> sim_pos = mean of matched-channel cosines, sim = sum of
  mean-over-batch all-pairs cosines, loss = -log(sim_pos/sim)/C/N.

Key algebra:
  * softmax denominator AND the max-subtraction cancel under l2
    normalization (S/W stays in [-0.7, 0.7] here, so exp needs no
    stabilization), so each projection only needs e = exp(S/W) and
    q = e/||e||; the reference's 1e-8 norm clamp never binds.
  * ||e||^2 = sum exp(2S/W) (second Exp pass + accum_out) and
    1/||e|| = exp(-0.5 ln ||e||^2), so with Copy the ACT engine only uses
    functions from ONE table set (natural_log_exp_and_others; the greedy
    set selector is steered there by _patch_act_tables) -> a single
    ACT_TABLE_LOAD, fully hidden under the DMA window.
  * sum_ij dot(qo_i, qt_j) = dot(sum_i qo_i, sum_j qt_j): the CxC pair
    matrix is never materialized. The per-map 1/||e|| factors are folded
    into the channel-membership matrices (g0, gt) and the matched-dot
    scalars; one [128x16x256] matmul per tensor yields both segments'
    channel sums (cross-segment junk is masked before the B reduction).
  * on-device reduction to 3 scalars per core (A = matched-cosine sums per
    segment, B = sum_n U.V); host all-reduces and takes the log.

Schedule: per-engine FIFO order arranged by expected data readiness; sync
queue: o chunks, the two small-descriptor tail DMAs, then t chunks. DMA
partition dim must be exactly 128 (anything else collapses the descriptor
spray onto 4 SDMA engines). Early chunks are 32 h-rows (16KB descriptors,
which avoid the SDMA-engine-15 slow-descriptor penalty); t tapers to
16/8/4/4-row chunks so the final DVE reduce off the last bytes is short.
No SWDGE/gpsimd DMAs; consts and the tail scatter ride the scalar HWDGE
queue. Per chunk: DVE segmented reduce (row sums) + PE transpose-
accumulate (col sums); all PSUM->SBUF copies are ACT Copy; the o-tensor's
softmax, its U-channel sums, and all tail work overlap t's DMA window.
"""

import contextlib
import ctypes
import sys
import types
from contextlib import ExitStack

import numpy as np

import concourse.bacc as bacc
import concourse.hw_specs as hw_specs
import concourse.tile as tile
from concourse import mybir
from concourse.bass_utils import run_bass_kernel_spmd

F32 = mybir.dt.float32
AX = mybir.AxisListType
ACT = mybir.ActivationFunctionType

N, C, H, W = 64, 17, 128, 128
NCORES = 8
NLOC = N // NCORES           # 8 batch entries per core
MAPS = NLOC * C              # 136 maps per tensor per core
MAIN = 128                   # maps in the main batch
TAIL = MAPS - MAIN           # 8 maps in the tail
ROWS_O = [16, 16, 32, 32, 32]          # small first chunks: early DVE start
ROWS_T = [32, 32, 16, 16, 16, 8, 4, 2, 2]  # taper: tiny final reduces


def _patch_act_tables():
    """Steer the greedy ACT table-set selector to the one set that holds
    every function this kernel uses (exp, ln, copy), so only one
    ACT_TABLE_LOAD is ever emitted."""
    orig = hw_specs.get_activation_tables
    strip = {ACT.Exp, ACT.Ln, ACT.Copy, ACT.Identity}

    def patched(arch):
        tabs = orig(arch)
        return {
            name: (fns if name == "natural_log_exp_and_others" else fns - strip)
            for name, fns in tabs.items()
        }

    bacc.get_activation_tables = patched


def _install_ntff_hook():
    """Provide antenv.axon_hooks if the image lacks it (needed only when
    run_bass_kernel_spmd is called with trace=True; harmless otherwise)."""
    if "antenv.axon_hooks" in sys.modules:
        return
    so_path = "/opt/axon/libaxon_pjrt.so"
    hook = None
    try:
        lib = ctypes.CDLL(so_path)
        if hasattr(lib, "axon_start_nrt_profile"):
            lib.axon_start_nrt_profile.argtypes = [
                ctypes.POINTER(ctypes.c_int64),
                ctypes.c_size_t,
            ]
            lib.axon_start_nrt_profile.restype = ctypes.c_int64
            lib.axon_stop_nrt_profile.argtypes = [ctypes.c_char_p]
            lib.axon_stop_nrt_profile.restype = ctypes.c_int64

            @contextlib.contextmanager
            def _hook(output_dir, device_ids):
                import jax

                jax.devices()
                if device_ids:
                    ids = (ctypes.c_int64 * len(device_ids))(*device_ids)
                    rc = lib.axon_start_nrt_profile(ids, len(device_ids))
                else:
                    rc = lib.axon_start_nrt_profile(None, 0)
                if rc != 0:
                    raise RuntimeError(f"axon_start_nrt_profile rc={rc}")
                try:
                    yield
                finally:
                    n = lib.axon_stop_nrt_profile(str(output_dir).encode())
                    print(f"profile: {n} file(s) in {output_dir}", file=sys.stderr)

            hook = _hook
    except OSError:
        pass
    mod = types.ModuleType("antenv.axon_hooks")
    mod.get_axon_ntff_profile_hook = lambda: hook
    mod.set_axon_ntff_profile_hook = lambda h: None
    sys.modules["antenv.axon_hooks"] = mod


_patch_act_tables()
_install_ntff_hook()


def _body(tc, o_d, t_d, id_d, g0_d, gt_d, on_d, res_d):
    nc = tc.nc
    with ExitStack() as ctx:
        consts = ctx.enter_context(tc.tile_pool(name="consts", bufs=1))
        chunks = ctx.enter_context(tc.tile_pool(name="chunks", bufs=1))
        tailp = ctx.enter_context(tc.tile_pool(name="tailp", bufs=1))
        projp = ctx.enter_context(tc.tile_pool(name="projp", bufs=1))
        workp = ctx.enter_context(tc.tile_pool(name="workp", bufs=1))
        outp = ctx.enter_context(tc.tile_pool(name="outp", bufs=1))
        # PSUM: 8 distinct tiles = 8 banks, no slot rotation (slot reuse
        # with concurrent PE traffic wedges the device: NRT status 101).
        accps = ctx.enter_context(tc.tile_pool(name="accps", bufs=1, space="PSUM"))

        # ---- consts on the scalar HWDGE queue ----
        ident = consts.tile([128, 128], F32)
        nc.scalar.dma_start(ident[:], id_d)
        g0 = consts.tile([MAIN, NLOC], F32)
        nc.scalar.dma_start(g0[:], g0_d)
        gt = consts.tile([TAIL, NLOC], F32)
        nc.scalar.dma_start(gt[:], gt_d)
        ones = consts.tile([128, 1], F32)
        nc.scalar.dma_start(ones[:], on_d)

        # ---- sync-queue DMAs: o chunks, tails, t chunks ----
        chunk_tiles = {0: [], 1: []}
        r0s = {0: 0, 1: 0}

        def issue_chunks(ti, x_d, rows, lo, hi):
            for c in range(lo, hi):
                r = rows[c]
                r0 = r0s[ti]
                ck = chunks.tile([MAIN, r * W], F32, name=f"chunk{ti}_{c}")
                nc.sync.dma_start(ck[:], x_d[0:MAIN, r0 : r0 + r, :])
                chunk_tiles[ti].append((ck, r0, r))
                r0s[ti] += r

        tail2d = tailp.tile([128, 2 * TAIL * W], F32)
        tv = tail2d.rearrange("p (m w) -> p m w", w=W)
        nc.sync.dma_start(tv[:, 0:TAIL, :], o_d[MAIN:MAPS].rearrange("m h w -> h m w"))
        nc.sync.dma_start(
            tv[:, TAIL : 2 * TAIL, :], t_d[MAIN:MAPS].rearrange("m h w -> h m w")
        )
        issue_chunks(0, o_d, ROWS_O, 0, len(ROWS_O))
        issue_chunks(1, t_d, ROWS_T, 0, len(ROWS_T))

        # PSUM tiles (8 banks)
        wt_o = accps.tile([128, MAIN], F32)
        wt_t = accps.tile([128, MAIN], F32)
        wb_o = accps.tile([MAIN, 128], F32)
        wb_t = accps.tile([MAIN, 128], F32)
        tlA = accps.tile([65, 512], F32)
        tlB = accps.tile([65, 512], F32)
        # U8/V8: per-n channel sums, h-segment in cols 0:128, w-segment in
        # cols 128:256. U8 also hosts A in [0:2, 256] and B in [0:1, 257].
        U8 = accps.tile([NLOC, 512], F32)
        V8 = accps.tile([NLOC, 512], F32)

        proj_o = projp.tile([MAIN, 2 * W], F32)
        proj_t = projp.tile([MAIN, 2 * W], F32)
        eo = projp.tile([MAIN, 2 * W], F32)
        et = projp.tile([MAIN, 2 * W], F32)
        To = tailp.tile([TAIL, 2 * W], F32)
        Tt = tailp.tile([TAIL, 2 * W], F32)
        To_e = tailp.tile([TAIL, 2 * W], F32)
        Tt_e = tailp.tile([TAIL, 2 * W], F32)
        # ssq/rn cols: 0:2 = o(h,w), 2:4 = tail-o, 4:6 = tail-t, 6:8 = t
        ssq = workp.tile([128, 8], F32)
        lssq = workp.tile([128, 8], F32)
        rn = workp.tile([128, 8], F32)
        dump = workp.tile([MAIN, 2 * W], F32)
        dumpP = workp.tile([MAIN, 2 * W], F32)
        dumpT = tailp.tile([TAIL, 2 * W], F32)
        dumpS = tailp.tile([TAIL, 2 * W], F32)

        def seg_exp(P, proj, e, dmp, col):
            """e = exp(S/W) for one 128-wide segment; ssq col = sum exp(2S/W)."""
            nc.scalar.activation(e, proj, ACT.Exp, scale=1.0 / W)
            nc.scalar.activation(
                dmp, proj, ACT.Exp, scale=2.0 / W,
                accum_out=ssq[0:P, col : col + 1],
            )

        def do_chunk(ti, proj, wt, c, act_red=False):
            ck, r0, r = chunk_tiles[ti][c]
            cv = ck.rearrange("p (h w) -> p h w", w=W)
            nc.vector.reduce_sum(proj[:, r0 : r0 + r], cv, axis=AX.X)
            nch = len(chunk_tiles[ti])
            for j in range(r):
                nc.tensor.matmul(
                    wt[:],
                    cv[:, j, :],
                    ident[:],
                    is_transpose=True,
                    start=(c == 0 and j == 0),
                    stop=(c == nch - 1 and j == r - 1),
                )

        # tail w-marginals: four [1,512] ones-matmul column-sum rows
        tl_rows = [(tlA, 32, 0), (tlA, 64, 1), (tlB, 32, 2), (tlB, 64, 3)]
        for tl, base, kk in tl_rows:
            nc.tensor.matmul(
                tl[base : base + 1, :],
                ones[:],
                tail2d[:, kk * 512 : (kk + 1) * 512],
                skip_group_check=True,
            )
        R = tailp.tile([128, 2 * TAIL], F32)
        nc.vector.reduce_sum(R[:], tv, axis=AX.X)
        for i, (T, tl) in enumerate(((To, tlA), (Tt, tlB))):
            nc.tensor.matmul(
                tl[0:TAIL, 0:128],
                R[:, i * TAIL : (i + 1) * TAIL],
                ident[:],
                is_transpose=True,
                skip_group_check=True,
            )
            nc.scalar.copy(T[:, 0:W], tl[0:TAIL, 0:128])
        srow = tailp.tile([65, 512 * 4], F32)
        for tl, base, kk in tl_rows:
            nc.scalar.copy(srow[base : base + 1, kk * 512 : (kk + 1) * 512],
                           tl[base : base + 1, :])
        scat = [(To, 0, 32, 0), (To, 4, 64, 1), (Tt, 0, 32, 2), (Tt, 4, 64, 3)]
        for T, m0, base, kk in scat:
            nc.scalar.dma_start(
                T[m0 : m0 + 4, W : 2 * W],
                srow[base : base + 1, kk * 512 : (kk + 1) * 512],
            )
        for i, (T, T_e) in enumerate(((To, To_e), (Tt, Tt_e))):
            for s in range(2):
                seg_exp(
                    TAIL, T[:, s * W : (s + 1) * W], T_e[:, s * W : (s + 1) * W],
                    dumpT[:, s * W : (s + 1) * W], 2 + 2 * i + s,
                )

        # tail rn (cols 2:6), matched dots and g-folds — all in the DVE
        # bubbles of o's arrival-paced phase
        nc.scalar.activation(lssq[:, 2:6], ssq[:, 2:6], ACT.Ln)
        nc.scalar.activation(rn[:, 2:6], lssq[:, 2:6], ACT.Exp, scale=-0.5)
        post_seg = outp.tile([TAIL, 2], F32)
        nc.vector.tensor_mul(dumpS[:], To_e[:], Tt_e[:])
        nc.vector.reduce_sum(
            post_seg[:], dumpS.rearrange("p (s w) -> p s w", w=W), axis=AX.X
        )
        nc.vector.tensor_mul(post_seg[:], post_seg[:], rn[0:TAIL, 2:4])
        nc.vector.tensor_mul(post_seg[:], post_seg[:], rn[0:TAIL, 4:6])
        gtc_o = outp.tile([TAIL, 2 * NLOC], F32)
        gtc_t = outp.tile([TAIL, 2 * NLOC], F32)
        for s in range(2):
            nc.vector.tensor_scalar_mul(
                gtc_o[:, s * NLOC : (s + 1) * NLOC], gt[:], rn[0:TAIL, 2 + s : 3 + s]
            )
            nc.vector.tensor_scalar_mul(
                gtc_t[:, s * NLOC : (s + 1) * NLOC], gt[:], rn[0:TAIL, 4 + s : 5 + s]
            )

        # ---- all of o: chunks, then softmax-exp (tail data arrives after
        # o's bytes, so all tail compute comes after o's in engine FIFOs) ----
        for c in range(len(chunk_tiles[0])):
            do_chunk(0, proj_o, wt_o, c)
        seg_exp(MAIN, proj_o[:, 0:W], eo[:, 0:W], dump[:, 0:W], 0)
        wts_o = workp.tile([128, MAIN], F32)
        nc.scalar.copy(wts_o[:], wt_o[:])
        nc.tensor.matmul(wb_o[:], wts_o[:], ident[:], is_transpose=True)
        nc.scalar.copy(proj_o[:, W : 2 * W], wb_o[:])
        seg_exp(MAIN, proj_o[:, W : 2 * W], eo[:, W : 2 * W], dump[:, W : 2 * W], 1)

        # ---- o/tail finish, all hidden before t's compute ----
        # rn for o (cols 0:2; tail cols were finished early)
        nc.scalar.activation(lssq[:, 0:2], ssq[:, 0:2], ACT.Ln)
        nc.scalar.activation(rn[:, 0:2], lssq[:, 0:2], ACT.Exp, scale=-0.5)
        gsc_o = outp.tile([MAIN, 2 * NLOC], F32)
        for s in range(2):
            nc.vector.tensor_scalar_mul(
                gsc_o[:, s * NLOC : (s + 1) * NLOC], g0[:], rn[0:MAIN, s : s + 1]
            )
        for s in range(2):
            nc.tensor.matmul(
                U8[:, s * W : (s + 1) * W], gsc_o[:, s * NLOC : (s + 1) * NLOC],
                eo[:, s * W : (s + 1) * W],
                start=True, stop=False, skip_group_check=True)
            nc.tensor.matmul(
                U8[:, s * W : (s + 1) * W], gtc_o[:, s * NLOC : (s + 1) * NLOC],
                To_e[:, s * W : (s + 1) * W],
                start=False, stop=True, skip_group_check=True)
        Us = outp.tile([NLOC, 2 * W], F32)
        nc.scalar.copy(Us[:], U8[:, 0 : 2 * W])

        # ---- t chunks (the 8/4-row chunks' row-sums go to the otherwise
        # idle GPSIMD engine: when engine-15's DMA backlog releases the last
        # sems in a burst, DVE and GPSIMD drain the reduces in parallel) ----
        for c in range(len(chunk_tiles[1])):
            do_chunk(1, proj_t, wt_t, c)

        # ---- t finish: w-segment first (wb_t is ready before the last
        # h-row reduces land), then h-segment; V matmuls per segment ----
        wts_t = workp.tile([128, MAIN], F32)
        nc.scalar.copy(wts_t[:], wt_t[:])
        pos_seg = outp.tile([MAIN, 2], F32)
        gsc_t = outp.tile([MAIN, 2 * NLOC], F32)
        nc.tensor.matmul(wb_t[:], wts_t[:], ident[:], is_transpose=True)
        # w-segment
        nc.scalar.copy(proj_t[:, W : 2 * W], wb_t[:])
        seg_exp(MAIN, proj_t[:, W : 2 * W], et[:, W : 2 * W], dump[:, W : 2 * W], 7)
        nc.scalar.activation(lssq[:, 7:8], ssq[:, 7:8], ACT.Ln)
        nc.scalar.activation(rn[:, 7:8], lssq[:, 7:8], ACT.Exp, scale=-0.5)
        nc.vector.tensor_mul(dumpP[:, W : 2 * W], eo[:, W : 2 * W], et[:, W : 2 * W])
        nc.vector.reduce_sum(pos_seg[:, 1:2], dumpP[:, W : 2 * W], axis=AX.X)
        nc.vector.tensor_mul(pos_seg[:, 1:2], pos_seg[:, 1:2], rn[0:MAIN, 1:2])
        nc.vector.tensor_mul(pos_seg[:, 1:2], pos_seg[:, 1:2], rn[0:MAIN, 7:8])
        nc.vector.tensor_scalar_mul(gsc_t[:, NLOC : 2 * NLOC], g0[:], rn[0:MAIN, 7:8])
        nc.tensor.matmul(V8[:, W : 2 * W], gsc_t[:, NLOC : 2 * NLOC],
                         et[:, W : 2 * W],
                         start=True, stop=False, skip_group_check=True)
        nc.tensor.matmul(V8[:, W : 2 * W], gtc_t[:, NLOC : 2 * NLOC],
                         Tt_e[:, W : 2 * W],
                         start=False, stop=True, skip_group_check=True)
        # h-segment
        seg_exp(MAIN, proj_t[:, 0:W], et[:, 0:W], dump[:, 0:W], 6)
        nc.scalar.activation(lssq[:, 6:7], ssq[:, 6:7], ACT.Ln)
        nc.scalar.activation(rn[:, 6:7], lssq[:, 6:7], ACT.Exp, scale=-0.5)
        nc.vector.tensor_mul(dumpP[:, 0:W], eo[:, 0:W], et[:, 0:W])
        nc.vector.reduce_sum(pos_seg[:, 0:1], dumpP[:, 0:W], axis=AX.X)
        nc.vector.tensor_mul(pos_seg[:, 0:1], pos_seg[:, 0:1], rn[0:MAIN, 0:1])
        nc.vector.tensor_mul(pos_seg[:, 0:1], pos_seg[:, 0:1], rn[0:MAIN, 6:7])
        nc.vector.tensor_scalar_mul(gsc_t[:, 0:NLOC], g0[:], rn[0:MAIN, 6:7])
        nc.tensor.matmul(V8[:, 0:W], gsc_t[:, 0:NLOC], et[:, 0:W],
                         start=True, stop=False, skip_group_check=True)
        nc.tensor.matmul(V8[:, 0:W], gtc_t[:, 0:NLOC], Tt_e[:, 0:W],
                         start=False, stop=True, skip_group_check=True)

        # A (per-segment matched-cosine sums) into U16[0:2, 256]
        nc.tensor.matmul(
            U8[0:2, 256:257], pos_seg[:], ones[0:MAIN, 0:1],
            start=True, stop=False, skip_group_check=True,
        )
        nc.tensor.matmul(
            U8[0:2, 256:257], post_seg[:], ones[0:TAIL, 0:1],
            start=False, stop=True, skip_group_check=True,
        )

        # B = sum_n U.V into U8[0:1, 257]: one full-row reduce covers both
        # segments (no junk in the [8, 256] layout)
        uv = outp.tile([NLOC, 2 * W], F32)
        nc.vector.tensor_mul(uv[:], Us[:], V8[:, 0 : 2 * W])
        uvs = outp.tile([NLOC, 1], F32)
        nc.vector.reduce_sum(uvs[:], uv[:], axis=AX.X)
        nc.tensor.matmul(
            U8[0:1, 257:258], uvs[:], ones[0:NLOC, 0:1], skip_group_check=True
        )

        res_s = outp.tile([2, 2], F32)
        nc.scalar.copy(res_s[:], U8[0:2, 256:258])
        nc.sync.dma_start(res_d, res_s[:])


def _build_nc():
    nc = bacc.Bacc("TRN2", target_bir_lowering=False, debug=False)
    o_d = nc.dram_tensor("o", [MAPS, H, W], F32, kind="ExternalInput").ap()
    t_d = nc.dram_tensor("t", [MAPS, H, W], F32, kind="ExternalInput").ap()
    id_d = nc.dram_tensor("ident", [128, 128], F32, kind="ExternalInput").ap()
    g0_d = nc.dram_tensor("g0", [MAIN, NLOC], F32, kind="ExternalInput").ap()
    gt_d = nc.dram_tensor("gt", [TAIL, NLOC], F32, kind="ExternalInput").ap()
    on_d = nc.dram_tensor("ones", [128, 1], F32, kind="ExternalInput").ap()
    res_d = nc.dram_tensor("res", [2, 2], F32, kind="ExternalOutput").ap()
    with tile.TileContext(nc) as tc:
        _body(tc, o_d, t_d, id_d, g0_d, gt_d, on_d, res_d)
    nc.compile()
    return nc


_NC = None


def _get_nc():
    global _NC
    if _NC is None:
        _NC = _build_nc()
    return _NC


_IDENT = np.eye(128, dtype=np.float32)
_G0 = np.zeros((MAIN, NLOC), np.float32)
_G0[np.arange(MAIN), np.arange(MAIN) // C] = 1.0
_GT = np.zeros((TAIL, NLOC), np.float32)
_GT[np.arange(TAIL), (MAIN + np.arange(TAIL)) // C] = 1.0
_ONES = np.ones((128, 1), np.float32)


def _make_in_maps(output, target):
    in_maps = []
    for i in range(NCORES):
        o = np.ascontiguousarray(output[i * NLOC : (i + 1) * NLOC]).reshape(MAPS, H, W)
        t = np.ascontiguousarray(target[i * NLOC : (i + 1) * NLOC]).reshape(MAPS, H, W)
        in_maps.append(
            {"o": o, "t": t, "ident": _IDENT, "g0": _G0, "gt": _GT,
             "ones": _ONES}
        )
    return in_maps


def _finish(results):
    A = 0.0
    B = 0.0
    for r in results:
        res = r["res"].astype(np.float64)
        A += res[0, 0] + res[1, 0]
        B += res[0, 1]
    # sim_pos = 0.5*A/(N*C); sim = 0.5*B/N; loss = -log(sim_pos/sim)/(C*N)
    loss = -np.log(A / (C * B)) / (C * N)
    return np.float32(loss)


def kernel(output, target):
    output = np.asarray(output, dtype=np.float32)
    target = np.asarray(target, dtype=np.float32)
    nc = _get_nc()
    res = run_bass_kernel_spmd(nc, _make_in_maps(output, target), list(range(NCORES)))
    return _finish(res.results)


def profile(output, target):
    """Run once with NTFF tracing; returns max per-core HW exec time in ns."""
    output = np.asarray(output, dtype=np.float32)
    target = np.asarray(target, dtype=np.float32)
    nc = _get_nc()
    res = run_bass_kernel_spmd(
        nc, _make_in_maps(output, target), list(range(NCORES)), trace=True
    )
    return res.exec_time_ns
